# revision 1
# baseline (speedup 1.0000x reference)
"""Self-contained Trainium2 Bass kernel for nn_Attention_26740466385723.

Full-input contract: kernel(**inputs) takes the unsharded numpy inputs and
returns the full [4, 1024, 1024] output.

Sharding (zero-collective): 8 cores = 4 batch x 2 sequence-halves. Each core
computes the attention output rows for half the sequence of one batch element.
The KV projection is duplicated within each batch pair (33% extra flops) which
avoids any inter-core collective.

Per-core device algorithm (all layouts chosen so no on-device transpose is
ever needed):
  qT  = Wq^T x_own^T        [inner, 512]   (Wq stationary, xT moving)
  kT  = Wk^T x_all^T        [inner, 1024]
  v   = x_all Wv            [1024, inner]  (xT stationary, Wv moving)
  rotary on qT/kT: qrotT = qT*cosT + (R2 @ qT)*sinT  (R2 block-diag rotation)
  per head h: scoresT[j, r] = sum_d kT[d,j] qT[d,r]
              P^T = exp(scoresT * 0.125)   (no max subtraction; scores~N(0,1))
              pv  = [v_h | 1]^T @ P^T -> [65, r], row 64 = softmax denominator
  A^T normalized via selector-matmul broadcast of 1/denom
  out = A Wo + bo          [512, 1024]    (A^T stationary, Wo moving)
"""

import sys
import os

if "/opt/trn_rl_repo" not in sys.path:
    sys.path.insert(0, "/opt/trn_rl_repo")

import numpy as np

HEADS = 16
DH = 64
B = 4
N = 1024
D = 1024
INNER = 1024
NCORES = 8
R = 512  # rows (query positions) per core
SCALE = DH ** -0.5  # 0.125

# matmul dtype: "bf16" (low power, FWL, 1 cyc/row), "f32r" (TF32-like,
# 1 cyc/row but power-throttles), or "f32" (exact, 4 cyc/row)
DTYPE_MODE = os.environ.get("BASS_ATTN_DTYPE", "bf16")

_CACHE = {}


def _build(dtype_mode: str, has_bias: bool = True):
    import concourse.bass as bass
    import concourse.mybir as mybir
    from concourse import bacc
    from concourse.tile import TileContext

    F32 = mybir.dt.float32
    # Tiles feeding the tensor engine are allocated in the matmul dtype so
    # producers (DMA/DVE/ACT) round on write, which the BIR verifier
    # requires for f32r and which enables FWL + low power for bf16.
    MDT = {"bf16": mybir.dt.bfloat16,
           "f32r": mybir.dt.float32r,
           "f32": mybir.dt.float32}[dtype_mode]

    def mm(ap):
        return ap

    nc = bacc.Bacc("TRN2", target_bir_lowering=False, debug=False,
                   num_devices=NCORES)

    xt = nc.dram_tensor("xt", [D, N], MDT, kind="ExternalInput")
    wq = nc.dram_tensor("wq", [D, INNER], MDT, kind="ExternalInput")
    wkv = nc.dram_tensor("wkv", [D, 2 * INNER], MDT, kind="ExternalInput")
    wo = nc.dram_tensor("wo", [INNER, D], MDT, kind="ExternalInput")
    bo = nc.dram_tensor("bo", [1, D], MDT, kind="ExternalInput")
    cosk = nc.dram_tensor("cosk", [128, N], F32, kind="ExternalInput")
    sink = nc.dram_tensor("sink", [128, N], F32, kind="ExternalInput")
    r2t = nc.dram_tensor("r2t", [128, 128], MDT, kind="ExternalInput")
    sel = nc.dram_tensor("sel", [HEADS, 8 * 128], MDT, kind="ExternalInput")
    ones1 = nc.dram_tensor("ones1", [1, 128], MDT, kind="ExternalInput")
    out = nc.dram_tensor("out", [R, D], F32, kind="ExternalOutput")
    # DRAM bounce to transpose the 16 softmax denominators from a
    # single-partition staging row into a [16, 512] partition-major tile.
    dbounce = nc.dram_tensor("dbounce", [HEADS, R], MDT)

    KC = D // 128  # 8 contraction chunks of 128
    VW = HEADS * (DH + 1)  # 1040: v columns with a ones column per head

    with TileContext(nc) as tc:
        with tc.tile_pool(name="persist", bufs=1) as persist:
            # Persistent SBUF tensors (live across phases):
            # qt 16K + kt 32K + v 36K = 84KB/partition
            qt_sb = persist.tile([128, KC, R], MDT)          # qT (rotated)
            kt_sb = persist.tile([128, KC, N], MDT)          # kT (rotated)
            v_sb = persist.tile([128, KC, VW], MDT)          # v + ones cols
            r2t_sb = persist.tile([128, 128], MDT)

            nc.sync.dma_start(out=r2t_sb[:], in_=r2t[:, :])

            # ones columns of v (column DH of each head's 65-wide group);
            # f32r memset fails the ISA check, so memset via an f32-typed
            # view there (1.0 is exact so no rounding is needed)
            vv = v_sb.rearrange("p c (h e) -> p c h e", e=DH + 1)
            ones_col = vv[:, :, :, DH:DH + 1]
            if MDT == mybir.dt.float32r:
                ones_col = ones_col.bitcast(F32)
            nc.vector.memset(ones_col, 1.0)

            # ---------------- Phase 1: QKV projections + rotary ----------
            # SBUF: xt 32K + cos 4K + sin 4K + weights 2x32K = 104KB
            with tc.tile_pool(name="ph1", bufs=1) as ph1, \
                 tc.tile_pool(name="wpool", bufs=2) as wpool, \
                 tc.tile_pool(name="rot_tmp", bufs=2) as rot_tmp, \
                 tc.tile_pool(name="ps_qkv", bufs=6, space="PSUM") as ps_qkv:

                xt_sb = ph1.tile([128, KC, N], MDT)
                cos_sb = ph1.tile([128, N], F32)
                sin_sb = ph1.tile([128, N], F32)
                # chunked loads, own-half columns first so the q matmuls
                # (which only read xt[:, 0:512]) start after ~2us; spread
                # across engine DMA queues so they run in parallel
                wq_sb = wpool.tile([128, KC, INNER], MDT, tag="w")
                for half in range(2):
                    lo, hi = half * 512, (half + 1) * 512
                    for k in range(KC):
                        nc.scalar.dma_start(
                            out=xt_sb[:, k, lo:hi],
                            in_=xt[k * 128:(k + 1) * 128, lo:hi])
                        if half == 0:
                            nc.sync.dma_start(
                                out=wq_sb[:, k, lo:hi],
                                in_=wq[k * 128:(k + 1) * 128, lo:hi])
                            nc.sync.dma_start(
                                out=wq_sb[:, k, hi:hi + 512],
                                in_=wq[k * 128:(k + 1) * 128, hi:hi + 512])
                    nc.scalar.dma_start(out=cos_sb[:, lo:hi],
                                        in_=cosk[:, lo:hi])
                    nc.scalar.dma_start(out=sin_sb[:, lo:hi],
                                        in_=sink[:, lo:hi])

                # swap adjacent partitions (2i <-> 2i+1) per 32-lane group;
                # the rotate-half sign lives in the host-signed sine input
                SWAP_MASK = [i ^ 1 for i in range(32)]
                rot_n = [0]

                def rotary(dst, cos_slc, sin_slc):
                    """dst (sbuf [128, 512]) holds fresh pre-rotary values;
                    apply dst = dst*cos + shuffle(dst)*sin_signed in place."""
                    rot_n[0] += 1
                    rt = rot_tmp.tile([128, 512], MDT,
                                      name=f"rt{rot_n[0]}", tag="rt")
                    nc.vector.stream_shuffle(rt[:], dst, mask=SWAP_MASK)
                    nc.vector.tensor_mul(rt[:], rt[:], sin_slc)
                    nc.vector.tensor_mul(dst, dst, cos_slc)
                    nc.vector.tensor_add(dst, dst, rt[:])

                # qT: out chunk c = [128 inner-cols, 512 own rows]
                for c in range(KC):
                    q_ps = ps_qkv.tile([128, 512], F32, tag="ps")
                    for k in range(KC):
                        nc.tensor.matmul(
                            q_ps[:],
                            mm(wq_sb[:, k, c * 128:(c + 1) * 128]),
                            mm(xt_sb[:, k, 0:R]),
                            start=(k == 0), stop=(k == KC - 1))
                    nc.vector.tensor_copy(qt_sb[:, c, :], q_ps[:])
                    rotary(qt_sb[:, c, :], cos_sb[:, 0:R], sin_sb[:, 0:R])

                # kT: out chunk c = [128 inner-cols, 1024 seq], 2 n-halves
                wk_sb = wpool.tile([128, KC, INNER], MDT, tag="w")
                for k in range(KC):
                    nc.sync.dma_start(
                        out=wk_sb[:, k, :],
                        in_=wkv[k * 128:(k + 1) * 128, 0:INNER])
                for c in range(KC):
                    for jh in range(2):
                        k_ps = ps_qkv.tile([128, 512], F32, tag="ps")
                        for k in range(KC):
                            nc.tensor.matmul(
                                k_ps[:],
                                mm(wk_sb[:, k, c * 128:(c + 1) * 128]),
                                mm(xt_sb[:, k, jh * 512:(jh + 1) * 512]),
                                start=(k == 0), stop=(k == KC - 1))
                        dst = kt_sb[:, c, jh * 512:(jh + 1) * 512]
                        nc.vector.tensor_copy(dst, k_ps[:])
                        rotary(dst, cos_sb[:, jh * 512:(jh + 1) * 512],
                               sin_sb[:, jh * 512:(jh + 1) * 512])

                # v: normal layout [seq, inner]; chunk m = seq rows,
                # n-half = 512 inner cols = 8 heads
                wv_sb = wpool.tile([128, KC, INNER], MDT, tag="w")
                for k in range(KC):
                    nc.sync.dma_start(
                        out=wv_sb[:, k, :],
                        in_=wkv[k * 128:(k + 1) * 128, INNER:2 * INNER])
                for m in range(KC):
                    for nh in range(2):
                        v_ps = ps_qkv.tile([128, 512], F32, tag="ps")
                        for k in range(KC):
                            nc.tensor.matmul(
                                v_ps[:],
                                mm(xt_sb[:, k, m * 128:(m + 1) * 128]),
                                mm(wv_sb[:, k, nh * 512:(nh + 1) * 512]),
                                start=(k == 0), stop=(k == KC - 1))
                        # scatter 8 heads of 64 cols into 65-strided groups
                        dst = vv[:, m, nh * 8:(nh + 1) * 8, 0:DH]
                        src = v_ps[:].rearrange("p (h e) -> p h e", e=DH)
                        nc.vector.tensor_copy(dst, src)

            # ------------- Phase 2: attention (2-deep pipelined) ----------
            # Heads are software-pipelined: scores for head h+2 are emitted
            # before PV of head h so the PE never waits on ACT's exp latency.
            # Softmax denominators are handled per head-pair: staged to
            # partition 0, bounced through DRAM into a [2, 512] tile, then a
            # K=2 selector matmul broadcasts 1/denom over the pair's 128
            # partitions and A^T is normalized in-loop.
            with tc.tile_pool(name="pt", bufs=3) as pt_pool, \
                 tc.tile_pool(name="ph2", bufs=1) as ph2, \
                 tc.tile_pool(name="stg", bufs=2) as stg_pool, \
                 tc.tile_pool(name="drp", bufs=2) as dr_pool, \
                 tc.tile_pool(name="wo_pool", bufs=2) as wo_pool, \
                 tc.tile_pool(name="out_st", bufs=4) as out_st:

                at_sb = ph2.tile([128, KC, R], MDT)  # A^T (attn out)
                sel_sb = ph2.tile([HEADS, 8 * 128], MDT)
                bo_sb = ph2.tile([1, D], MDT)
                ones1_sb = ph2.tile([1, 128], MDT)
                nc.sync.dma_start(out=sel_sb[:], in_=sel[:, :])
                nc.sync.dma_start(out=bo_sb[:], in_=bo[:, :])
                nc.sync.dma_start(out=ones1_sb[:], in_=ones1[:, :])

                # prefetch both Wo halves during the head loop
                wo_tiles = []
                for n in range(2):
                    wo_sb = wo_pool.tile([128, KC, 512], MDT,
                                         name=f"wo{n}", tag="woh")
                    nc.sync.dma_start(
                        out=wo_sb[:],
                        in_=wo[:, n * 512:(n + 1) * 512].rearrange(
                            "(c p) m -> p c m", p=128))
                    wo_tiles.append(wo_sb)

                with tc.tile_pool(name="ps_s", bufs=2, space="PSUM") as ps_s, \
                     tc.tile_pool(name="ps_pv", bufs=2,
                                  space="PSUM") as ps_pv:

                    pts = {}
                    stages = {}

                    def scores(h):
                        """scoresT + exp for head h -> pts[h]."""
                        c = h // 2
                        po = (h % 2) * 64
                        pt = pt_pool.tile([128, KC, R], MDT, name=f"pt{h}", tag="pt")
                        pts[h] = pt
                        for jp in range(KC // 2):  # pairs of j-chunks
                            s_ps = ps_s.tile([128, 2, 512], F32, tag="s")
                            for jj in range(2):
                                j = 2 * jp + jj
                                nc.tensor.matmul(
                                    s_ps[:, jj, :],
                                    mm(kt_sb[po:po + 64, c,
                                             j * 128:(j + 1) * 128]),
                                    mm(qt_sb[po:po + 64, c, :]),
                                    start=True, stop=True)
                            nc.scalar.activation(
                                out=pt[:, 2 * jp:2 * jp + 2, :], in_=s_ps[:],
                                func=mybir.ActivationFunctionType.Exp,
                                scale=SCALE)

                    def pv(h):
                        """PV for head h + denominator staging + per-pair
                        normalization of A^T."""
                        c = h // 2
                        po = (h % 2) * 64
                        pt = pts.pop(h)
                        pv_ps = ps_pv.tile([128, 512], F32, tag="pv")
                        for j in range(KC):
                            nc.tensor.matmul(
                                pv_ps[0:DH + 1, :],
                                mm(v_sb[:, j,
                                        h * (DH + 1):(h + 1) * (DH + 1)]),
                                mm(pt[:, j, :]),
                                start=(j == 0), stop=(j == KC - 1))
                        nc.vector.tensor_copy(at_sb[po:po + 64, c, :],
                                              pv_ps[0:DH, :])
                        if h % 2 == 0:
                            stages[c] = stg_pool.tile([1, 2, R], MDT,
                                                      name=f"stg{c}", tag="stg")
                        nc.vector.tensor_copy(stages[c][0:1, h % 2, :],
                                              pv_ps[DH:DH + 1, :])
                        if h % 2 == 1:
                            # pair complete: denoms -> DRAM -> [2,512] tile,
                            # reciprocal, broadcast, normalize chunk c
                            stage = stages.pop(c)
                            nc.sync.dma_start(
                                out=dbounce[2 * c:2 * c + 2, :].unsqueeze(0),
                                in_=stage[:])
                            dr = dr_pool.tile([2, 2, R], MDT,
                                              name=f"dr{c}", tag="dr")
                            nc.sync.dma_start(out=dr[:, 0, :],
                                              in_=dbounce[2 * c:2 * c + 2, :])
                            with nc.allow_low_precision(
                                    reason="bf16 softmax denom (tol 2e-2)"):
                                nc.vector.reciprocal(dr[:, 1, :], dr[:, 0, :])
                            b_ps = ps_pv.tile([128, 512], F32, tag="b")
                            nc.tensor.matmul(
                                b_ps[:],
                                mm(sel_sb[0:2, 0:128]),
                                mm(dr[:, 1, :]),
                                start=True, stop=True)
                            nc.vector.tensor_mul(at_sb[:, c, :],
                                                 at_sb[:, c, :], b_ps[:])

                    scores(0)
                    scores(1)
                    for h in range(HEADS):
                        if h + 2 < HEADS:
                            scores(h + 2)
                        pv(h)

                # ------------- Phase 3: output projection ----------------
                with tc.tile_pool(name="ps_f", bufs=4, space="PSUM") as ps_f:
                    for n in range(2):
                        wo_sb = wo_tiles[n]
                        for m in range(4):
                            f_ps = ps_f.tile([128, 512], F32)
                            for k in range(KC):
                                nc.tensor.matmul(
                                    f_ps[:],
                                    mm(at_sb[:, k, m * 128:(m + 1) * 128]),
                                    mm(wo_sb[:, k, :]),
                                    start=(k == 0),
                                    stop=(not has_bias and k == KC - 1))
                            if has_bias:
                                nc.tensor.matmul(
                                    f_ps[:], mm(ones1_sb[:]),
                                    mm(bo_sb[0:1, n * 512:(n + 1) * 512]),
                                    start=False, stop=True)
                            o_sb = out_st.tile([128, 512], F32)
                            nc.vector.tensor_copy(o_sb[:], f_ps[:])
                            nc.sync.dma_start(
                                out=out[m * 128:(m + 1) * 128,
                                        n * 512:(n + 1) * 512],
                                in_=o_sb[:])

    nc.compile()
    return nc


def _host_prep(x, rotary_emb, Wq, Wkv, Wo, bo, dtype_mode="f32"):
    """Build the per-core input maps."""
    if dtype_mode == "bf16":
        import ml_dtypes
        mnp = ml_dtypes.bfloat16
    else:
        mnp = np.float32
    x = np.asarray(x, dtype=np.float32)
    rotary_emb = np.asarray(rotary_emb, dtype=np.float32)
    Wq = np.ascontiguousarray(np.asarray(Wq, dtype=np.float32))
    Wkv = np.ascontiguousarray(np.asarray(Wkv, dtype=np.float32))
    Wo = np.ascontiguousarray(np.asarray(Wo, dtype=np.float32))
    bo_row = np.ascontiguousarray(np.asarray(bo, dtype=np.float32)[None, :])

    cosT = np.cos(rotary_emb).T.astype(np.float32)  # [64, 1024]
    sinT = np.sin(rotary_emb).T.astype(np.float32)
    cos2 = np.concatenate([cosT, cosT], axis=0)  # [128, n]
    sin2 = np.concatenate([sinT, sinT], axis=0)
    # rotate-half sign: rot[2i] = -x[2i+1], rot[2i+1] = +x[2i]; the device
    # only swaps lanes, so bake the sign into the sine rows
    sign = np.where(np.arange(128) % 2 == 0, -1.0, 1.0).astype(np.float32)
    sin2 = sin2 * sign[:, None]

    # R2^T: rot(v)[2i] = -v[2i+1], rot(v)[2i+1] = v[2i]
    R64 = np.zeros((DH, DH), dtype=np.float32)
    for i in range(DH // 2):
        R64[2 * i, 2 * i + 1] = -1.0
        R64[2 * i + 1, 2 * i] = 1.0
    R2 = np.zeros((128, 128), dtype=np.float32)
    R2[:DH, :DH] = R64
    R2[DH:, DH:] = R64
    r2t = np.ascontiguousarray(R2.T)

    sel = np.zeros((HEADS, 8 * 128), dtype=np.float32)
    for t in range(8):
        for p in range(128):
            sel[2 * t + p // 64, t * 128 + p] = 1.0

    ones1 = np.ones((1, 128), dtype=np.float32)

    in_maps = []
    for core in range(NCORES):
        b, half = divmod(core, 2)
        perm = np.concatenate([
            np.arange(half * R, (half + 1) * R),
            np.arange((1 - half) * R, (2 - half) * R)])
        xt = np.ascontiguousarray(x[b].T[:, perm])  # [D, N] own half first
        in_maps.append({
            "xt": xt.astype(mnp),
            "wq": Wq.astype(mnp),
            "wkv": Wkv.astype(mnp),
            "wo": Wo.astype(mnp),
            "bo": bo_row.astype(mnp),
            "cosk": np.ascontiguousarray(cos2[:, perm]),
            "sink": np.ascontiguousarray(sin2[:, perm]),
            "r2t": r2t.astype(mnp),
            "sel": sel.astype(mnp),
            "ones1": ones1.astype(mnp),
        })
    return in_maps


def _run(inputs, trace=False, trace_cores=None):
    from concourse.bass_utils import run_bass_kernel_spmd

    has_bias = bool(np.any(np.asarray(inputs["bo"])))
    key = ("nc", DTYPE_MODE, has_bias)
    if key not in _CACHE:
        _CACHE[key] = _build(DTYPE_MODE, has_bias=has_bias)
    nc = _CACHE[key]

    in_maps = _host_prep(dtype_mode=DTYPE_MODE, **inputs)
    res = run_bass_kernel_spmd(nc, in_maps, list(range(NCORES)),
                               trace=trace, trace_cores=trace_cores)
    out = np.empty((B, N, D), dtype=np.float32)
    for core in range(NCORES):
        b, half = divmod(core, 2)
        out[b, half * R:(half + 1) * R, :] = res.results[core]["out"]
    return out, res


def kernel(**inputs):
    out, _ = _run(inputs, trace=False)
    return out



# revision 20
# speedup vs baseline: 1.2325x; 1.2325x over previous
"""Self-contained Trainium2 Bass kernel for nn_Attention_26740466385723.

Full-input contract: kernel(**inputs) takes the unsharded numpy inputs and
returns the full [4, 1024, 1024] output.

Sharding (zero-collective): 8 cores = 4 batch x 2 sequence-halves. Each core
computes the attention output rows for half the sequence of one batch element.
The KV projection is duplicated within each batch pair (33% extra flops) which
avoids any inter-core collective.

Per-core device algorithm (all layouts chosen so no on-device transpose is
ever needed):
  qT  = Wq^T x_own^T        [inner, 512]   (Wq stationary, xT moving)
  kT  = Wk^T x_all^T        [inner, 1024]
  v   = x_all Wv            [1024, inner]  (xT stationary, Wv moving)
  rotary on qT/kT: qrotT = qT*cosT + (R2 @ qT)*sinT  (R2 block-diag rotation)
  per head h: scoresT[j, r] = sum_d kT[d,j] qT[d,r]
              P^T = exp(scoresT * 0.125)   (no max subtraction; scores~N(0,1))
              pv  = [v_h | 1]^T @ P^T -> [65, r], row 64 = softmax denominator
  A^T normalized via selector-matmul broadcast of 1/denom
  out = A Wo + bo          [512, 1024]    (A^T stationary, Wo moving)
"""

import sys
import os

if "/opt/trn_rl_repo" not in sys.path:
    sys.path.insert(0, "/opt/trn_rl_repo")

import numpy as np

HEADS = 16
DH = 64
B = 4
N = 1024
D = 1024
INNER = 1024
NCORES = 8
R = 512  # rows (query positions) per core
SCALE = DH ** -0.5  # 0.125

# matmul dtype: "bf16" (low power, FWL, 1 cyc/row), "f32r" (TF32-like,
# 1 cyc/row but power-throttles), or "f32" (exact, 4 cyc/row)
DTYPE_MODE = os.environ.get("BASS_ATTN_DTYPE", "bf16")

_CACHE = {}


def _build(dtype_mode: str, has_bias: bool = True):
    import concourse.bass as bass
    import concourse.mybir as mybir
    from concourse import bacc
    from concourse.tile import TileContext

    F32 = mybir.dt.float32
    # Tiles feeding the tensor engine are allocated in the matmul dtype so
    # producers (DMA/DVE/ACT) round on write, which the BIR verifier
    # requires for f32r and which enables FWL + low power for bf16.
    MDT = {"bf16": mybir.dt.bfloat16,
           "f32r": mybir.dt.float32r,
           "f32": mybir.dt.float32}[dtype_mode]

    def mm(ap):
        return ap

    nc = bacc.Bacc("TRN2", target_bir_lowering=False, debug=False,
                   num_devices=NCORES)

    xt = nc.dram_tensor("xt", [D, N], MDT, kind="ExternalInput")
    wq = nc.dram_tensor("wq", [D, INNER], MDT, kind="ExternalInput")
    wkv = nc.dram_tensor("wkv", [D, 2 * INNER], MDT, kind="ExternalInput")
    wo = nc.dram_tensor("wo", [INNER, D], MDT, kind="ExternalInput")
    bo = nc.dram_tensor("bo", [1, D], MDT, kind="ExternalInput")
    # cos/sin in the matmul dtype: 2-byte operands let the rotary
    # tensor_tensor ops hit the DVE 2x/4x perf modes
    cosk = nc.dram_tensor("cosk", [128, N], MDT, kind="ExternalInput")
    sink = nc.dram_tensor("sink", [128, N], MDT, kind="ExternalInput")
    sel = nc.dram_tensor("sel", [2, 128], MDT, kind="ExternalInput")
    ones1 = nc.dram_tensor("ones1", [1, 128], MDT, kind="ExternalInput")
    out = nc.dram_tensor("out", [R, D], F32, kind="ExternalOutput")

    KC = D // 128  # 8 contraction chunks of 128
    VW = HEADS * (DH + 1)  # 1040: v columns with a ones column per head

    with TileContext(nc) as tc:
        with tc.tile_pool(name="persist", bufs=1) as persist:
            # Persistent SBUF tensors (live across phases):
            # qt 16K + kt 32K + v 36K = 84KB/partition
            qt_sb = persist.tile([128, KC, R], MDT)          # qT (rotated)
            kt_sb = persist.tile([128, KC, N], MDT)          # kT (rotated)
            v_sb = persist.tile([128, KC, VW], MDT)          # v + ones cols

            # ones columns of v (column DH of each head's 65-wide group);
            # f32r memset fails the ISA check, so memset via an f32-typed
            # view there (1.0 is exact so no rounding is needed)
            vv = v_sb.rearrange("p c (h e) -> p c h e", e=DH + 1)
            ones_col = vv[:, :, :, DH:DH + 1]
            if MDT == mybir.dt.float32r:
                ones_col = ones_col.bitcast(F32)
            nc.vector.memset(ones_col, 1.0)

            # ---------------- Phase 1: QKV projections + rotary ----------
            # SBUF: xt 32K + cos 4K + sin 4K + weights 2x32K = 104KB
            with tc.tile_pool(name="ph1", bufs=1) as ph1, \
                 tc.tile_pool(name="wpool", bufs=2) as wpool, \
                 tc.tile_pool(name="rot_tmp", bufs=2) as rot_tmp, \
                 tc.tile_pool(name="ps_qkv", bufs=6, space="PSUM") as ps_qkv:

                xt_sb = ph1.tile([128, KC, N], MDT)
                cos_sb = ph1.tile([128, N], MDT)
                sin_sb = ph1.tile([128, N], MDT)
                # chunked loads, own-half columns first so the q matmuls
                # (which only read xt[:, 0:512]) start after ~2us; spread
                # across engine DMA queues so they run in parallel
                wq_sb = wpool.tile([128, KC, INNER], MDT, tag="w")
                for half in range(2):
                    lo, hi = half * 512, (half + 1) * 512
                    for k in range(KC):
                        nc.scalar.dma_start(
                            out=xt_sb[:, k, lo:hi],
                            in_=xt[k * 128:(k + 1) * 128, lo:hi])
                        if half == 0:
                            nc.sync.dma_start(
                                out=wq_sb[:, k, lo:hi],
                                in_=wq[k * 128:(k + 1) * 128, lo:hi])
                            nc.sync.dma_start(
                                out=wq_sb[:, k, hi:hi + 512],
                                in_=wq[k * 128:(k + 1) * 128, hi:hi + 512])
                    nc.scalar.dma_start(out=cos_sb[:, lo:hi],
                                        in_=cosk[:, lo:hi])
                    nc.scalar.dma_start(out=sin_sb[:, lo:hi],
                                        in_=sink[:, lo:hi])

                # swap adjacent partitions (2i <-> 2i+1) per 32-lane group;
                # the rotate-half sign lives in the host-signed sine input
                SWAP_MASK = [i ^ 1 for i in range(32)]
                rot_n = [0]

                def rotary(dst, cos_slc, sin_slc):
                    """dst (sbuf [128, 512]) holds fresh pre-rotary values;
                    apply dst = dst*cos + shuffle(dst)*sin_signed in place."""
                    rot_n[0] += 1
                    rt = rot_tmp.tile([128, 512], MDT,
                                      name=f"rt{rot_n[0]}", tag="rt")
                    nc.vector.stream_shuffle(rt[:], dst, mask=SWAP_MASK)
                    nc.vector.tensor_mul(rt[:], rt[:], sin_slc)
                    nc.vector.tensor_mul(dst, dst, cos_slc)
                    nc.vector.tensor_add(dst, dst, rt[:])

                # qT: out chunk c = [128 inner-cols, 512 own rows]
                for c in range(KC):
                    q_ps = ps_qkv.tile([128, 512], F32, tag="ps")
                    for k in range(KC):
                        nc.tensor.matmul(
                            q_ps[:],
                            mm(wq_sb[:, k, c * 128:(c + 1) * 128]),
                            mm(xt_sb[:, k, 0:R]),
                            start=(k == 0), stop=(k == KC - 1))
                    nc.scalar.copy(out=qt_sb[:, c, :], in_=q_ps[:])
                    rotary(qt_sb[:, c, :], cos_sb[:, 0:R], sin_sb[:, 0:R])

                # kT: out chunk c = [128 inner-cols, 1024 seq], 2 n-halves
                wk_sb = wpool.tile([128, KC, INNER], MDT, tag="w")
                for k in range(KC):
                    nc.sync.dma_start(
                        out=wk_sb[:, k, :],
                        in_=wkv[k * 128:(k + 1) * 128, 0:INNER])
                for c in range(KC):
                    for jh in range(2):
                        k_ps = ps_qkv.tile([128, 512], F32, tag="ps")
                        for k in range(KC):
                            nc.tensor.matmul(
                                k_ps[:],
                                mm(wk_sb[:, k, c * 128:(c + 1) * 128]),
                                mm(xt_sb[:, k, jh * 512:(jh + 1) * 512]),
                                start=(k == 0), stop=(k == KC - 1))
                        dst = kt_sb[:, c, jh * 512:(jh + 1) * 512]
                        nc.scalar.copy(out=dst, in_=k_ps[:])
                        rotary(dst, cos_sb[:, jh * 512:(jh + 1) * 512],
                               sin_sb[:, jh * 512:(jh + 1) * 512])

                # v: normal layout [seq, inner]; chunk m = seq rows,
                # n-half = 512 inner cols = 8 heads
                wv_sb = wpool.tile([128, KC, INNER], MDT, tag="w")
                for k in range(KC):
                    nc.sync.dma_start(
                        out=wv_sb[:, k, :],
                        in_=wkv[k * 128:(k + 1) * 128, INNER:2 * INNER])
                for m in range(KC):
                    for nh in range(2):
                        v_ps = ps_qkv.tile([128, 512], F32, tag="ps")
                        for k in range(KC):
                            nc.tensor.matmul(
                                v_ps[:],
                                mm(xt_sb[:, k, m * 128:(m + 1) * 128]),
                                mm(wv_sb[:, k, nh * 512:(nh + 1) * 512]),
                                start=(k == 0), stop=(k == KC - 1))
                        # scatter 8 heads of 64 cols into 65-strided groups
                        dst = vv[:, m, nh * 8:(nh + 1) * 8, 0:DH]
                        src = v_ps[:].rearrange("p (h e) -> p h e", e=DH)
                        nc.scalar.copy(out=dst, in_=src)

            # ------------- Phase 2: attention (2-deep pipelined) ----------
            # Heads are software-pipelined: scores for head h+2 are emitted
            # before PV of head h so the PE never waits on ACT's exp latency.
            # Softmax denominators are handled per head-pair: staged to
            # partition 0, bounced through DRAM into a [2, 512] tile, then a
            # K=2 selector matmul broadcasts 1/denom over the pair's 128
            # partitions and A^T is normalized in-loop.
            with tc.tile_pool(name="pt", bufs=3) as pt_pool, \
                 tc.tile_pool(name="ph2", bufs=1) as ph2, \
                 tc.tile_pool(name="stg", bufs=2) as stg_pool, \
                 tc.tile_pool(name="drp", bufs=2) as dr_pool, \
                 tc.tile_pool(name="wo_pool", bufs=2) as wo_pool, \
                 tc.tile_pool(name="out_st", bufs=4) as out_st:

                at_sb = ph2.tile([128, KC, R], MDT)  # A^T (attn out)
                # selector rows live on partition 64 (same base partition as
                # the psum denom row, required by the matmul operand check)
                sel_sb = ph2.tile([128, 2, 128], MDT)
                bo_sb = ph2.tile([1, D], MDT)
                ones1_sb = ph2.tile([1, 128], MDT)
                nc.sync.dma_start(out=sel_sb[DH:DH + 1, :, :],
                                  in_=sel[:, :].unsqueeze(0))
                nc.sync.dma_start(out=bo_sb[:], in_=bo[:, :])
                nc.sync.dma_start(out=ones1_sb[:], in_=ones1[:, :])

                # prefetch both Wo halves during the head loop
                wo_tiles = []
                for n in range(2):
                    wo_sb = wo_pool.tile([128, KC, 512], MDT,
                                         name=f"wo{n}", tag="woh")
                    nc.sync.dma_start(
                        out=wo_sb[:],
                        in_=wo[:, n * 512:(n + 1) * 512].rearrange(
                            "(c p) m -> p c m", p=128))
                    wo_tiles.append(wo_sb)

                with tc.tile_pool(name="ps_s", bufs=2, space="PSUM") as ps_s, \
                     tc.tile_pool(name="ps_pv", bufs=2,
                                  space="PSUM") as ps_pv:

                    pts = {}
                    stages = {}

                    def scores(h):
                        """scoresT + exp for head h -> pts[h]."""
                        c = h // 2
                        po = (h % 2) * 64
                        pt = pt_pool.tile([128, KC, R], MDT, name=f"pt{h}", tag="pt")
                        pts[h] = pt
                        for jp in range(KC // 2):  # pairs of j-chunks
                            s_ps = ps_s.tile([128, 2, 512], F32, tag="s")
                            for jj in range(2):
                                j = 2 * jp + jj
                                nc.tensor.matmul(
                                    s_ps[:, jj, :],
                                    mm(kt_sb[po:po + 64, c,
                                             j * 128:(j + 1) * 128]),
                                    mm(qt_sb[po:po + 64, c, :]),
                                    start=True, stop=True)
                            nc.scalar.activation(
                                out=pt[:, 2 * jp:2 * jp + 2, :], in_=s_ps[:],
                                func=mybir.ActivationFunctionType.Exp,
                                scale=SCALE)

                    def pv(h):
                        """PV for head h + denominator staging + per-pair
                        normalization of A^T."""
                        c = h // 2
                        po = (h % 2) * 64
                        pt = pts.pop(h)
                        pv_ps = ps_pv.tile([128, 512], F32, tag="pv")
                        for j in range(KC):
                            nc.tensor.matmul(
                                pv_ps[0:DH + 1, :],
                                mm(v_sb[:, j,
                                        h * (DH + 1):(h + 1) * (DH + 1)]),
                                mm(pt[:, j, :]),
                                start=(j == 0), stop=(j == KC - 1))
                        nc.vector.tensor_copy(at_sb[po:po + 64, c, :],
                                              pv_ps[0:DH, :])
                        if h % 2 == 0:
                            stages[c] = stg_pool.tile([128, 2, R], F32,
                                                      name=f"stg{c}", tag="stg")
                        # 1/denom straight off the psum denom row; stays on
                        # partition 64 (no cross-partition moves)
                        with nc.allow_low_precision(reason="softmax denom"):
                            nc.vector.reciprocal(
                                stages[c][DH:DH + 1, h % 2, :],
                                pv_ps[DH:DH + 1, :])
                        if h % 2 == 1:
                            # pair complete: cast to matmul dtype, broadcast
                            # each head's 1/denom over its 64 partitions via
                            # two accumulating K=1 selector matmuls,
                            # normalize chunk c
                            stage = stages.pop(c)
                            drb = dr_pool.tile([128, 2, R], MDT,
                                               name=f"dr{c}", tag="dr")
                            nc.vector.tensor_copy(drb[DH:DH + 1, :, :],
                                                  stage[DH:DH + 1, :, :])
                            b_ps = ps_pv.tile([128, 512], F32, tag="b")
                            nc.tensor.matmul(
                                b_ps[:],
                                mm(sel_sb[DH:DH + 1, 0, :]),
                                mm(drb[DH:DH + 1, 0, :]),
                                start=True, stop=False)
                            nc.tensor.matmul(
                                b_ps[:],
                                mm(sel_sb[DH:DH + 1, 1, :]),
                                mm(drb[DH:DH + 1, 1, :]),
                                start=False, stop=True)
                            nc.vector.tensor_mul(at_sb[:, c, :],
                                                 at_sb[:, c, :], b_ps[:])

                    scores(0)
                    scores(1)
                    for h in range(HEADS):
                        if h + 2 < HEADS:
                            scores(h + 2)
                        pv(h)

                # ------------- Phase 3: output projection ----------------
                with tc.tile_pool(name="ps_f", bufs=4, space="PSUM") as ps_f:
                    for n in range(2):
                        wo_sb = wo_tiles[n]
                        for m in range(4):
                            f_ps = ps_f.tile([128, 512], F32)
                            for k in range(KC):
                                nc.tensor.matmul(
                                    f_ps[:],
                                    mm(at_sb[:, k, m * 128:(m + 1) * 128]),
                                    mm(wo_sb[:, k, :]),
                                    start=(k == 0),
                                    stop=(not has_bias and k == KC - 1))
                            if has_bias:
                                nc.tensor.matmul(
                                    f_ps[:], mm(ones1_sb[:]),
                                    mm(bo_sb[0:1, n * 512:(n + 1) * 512]),
                                    start=False, stop=True)
                            o_sb = out_st.tile([128, 512], F32)
                            nc.scalar.copy(out=o_sb[:], in_=f_ps[:])
                            nc.sync.dma_start(
                                out=out[m * 128:(m + 1) * 128,
                                        n * 512:(n + 1) * 512],
                                in_=o_sb[:])

    nc.compile()
    return nc


def _host_prep(x, rotary_emb, Wq, Wkv, Wo, bo, dtype_mode="f32"):
    """Build the per-core input maps."""
    if dtype_mode == "bf16":
        import ml_dtypes
        mnp = ml_dtypes.bfloat16
    else:
        mnp = np.float32
    x = np.asarray(x, dtype=np.float32)
    rotary_emb = np.asarray(rotary_emb, dtype=np.float32)
    Wq = np.ascontiguousarray(np.asarray(Wq, dtype=np.float32))
    Wkv = np.ascontiguousarray(np.asarray(Wkv, dtype=np.float32))
    Wo = np.ascontiguousarray(np.asarray(Wo, dtype=np.float32))
    bo_row = np.ascontiguousarray(np.asarray(bo, dtype=np.float32)[None, :])

    cosT = np.cos(rotary_emb).T.astype(np.float32)  # [64, 1024]
    sinT = np.sin(rotary_emb).T.astype(np.float32)
    cos2 = np.concatenate([cosT, cosT], axis=0)  # [128, n]
    sin2 = np.concatenate([sinT, sinT], axis=0)
    # rotate-half sign: rot[2i] = -x[2i+1], rot[2i+1] = +x[2i]; the device
    # only swaps lanes, so bake the sign into the sine rows
    sign = np.where(np.arange(128) % 2 == 0, -1.0, 1.0).astype(np.float32)
    sin2 = sin2 * sign[:, None]

    # selector rows: head-even -> partitions 0..63, head-odd -> 64..127
    sel = np.zeros((2, 128), dtype=np.float32)
    sel[0, 0:64] = 1.0
    sel[1, 64:128] = 1.0

    ones1 = np.ones((1, 128), dtype=np.float32)

    in_maps = []
    for core in range(NCORES):
        b, half = divmod(core, 2)
        perm = np.concatenate([
            np.arange(half * R, (half + 1) * R),
            np.arange((1 - half) * R, (2 - half) * R)])
        xt = np.ascontiguousarray(x[b].T[:, perm])  # [D, N] own half first
        in_maps.append({
            "xt": xt.astype(mnp),
            "wq": Wq.astype(mnp),
            "wkv": Wkv.astype(mnp),
            "wo": Wo.astype(mnp),
            "bo": bo_row.astype(mnp),
            "cosk": np.ascontiguousarray(cos2[:, perm]).astype(mnp),
            "sink": np.ascontiguousarray(sin2[:, perm]).astype(mnp),
            "sel": sel.astype(mnp),
            "ones1": ones1.astype(mnp),
        })
    return in_maps


def _run(inputs, trace=False, trace_cores=None):
    from concourse.bass_utils import run_bass_kernel_spmd

    has_bias = bool(np.any(np.asarray(inputs["bo"])))
    key = ("nc", DTYPE_MODE, has_bias)
    if key not in _CACHE:
        _CACHE[key] = _build(DTYPE_MODE, has_bias=has_bias)
    nc = _CACHE[key]

    in_maps = _host_prep(dtype_mode=DTYPE_MODE, **inputs)
    res = run_bass_kernel_spmd(nc, in_maps, list(range(NCORES)),
                               trace=trace, trace_cores=trace_cores)
    out = np.empty((B, N, D), dtype=np.float32)
    for core in range(NCORES):
        b, half = divmod(core, 2)
        out[b, half * R:(half + 1) * R, :] = res.results[core]["out"]
    return out, res


def kernel(**inputs):
    out, _ = _run(inputs, trace=False)
    return out



# revision 24
# speedup vs baseline: 1.3742x; 1.1150x over previous
"""Self-contained Trainium2 Bass kernel for nn_Attention_26740466385723.

Full-input contract: kernel(**inputs) takes the unsharded numpy inputs and
returns the full [4, 1024, 1024] output.

Sharding (zero-collective): 8 cores = 4 batch x 2 sequence-halves. Each core
computes the attention output rows for half the sequence of one batch element.
The KV projection is duplicated within each batch pair (33% extra flops) which
avoids any inter-core collective.

Per-core device algorithm (all layouts chosen so no on-device transpose is
ever needed):
  qT  = Wq^T x_own^T        [inner, 512]   (Wq stationary, xT moving)
  kT  = Wk^T x_all^T        [inner, 1024]
  v   = x_all Wv            [1024, inner]  (xT stationary, Wv moving)
  rotary on qT/kT: qrotT = qT*cosT + (R2 @ qT)*sinT  (R2 block-diag rotation)
  per head h: scoresT[j, r] = sum_d kT[d,j] qT[d,r]
              P^T = exp(scoresT * 0.125)   (no max subtraction; scores~N(0,1))
              pv  = [v_h | 1]^T @ P^T -> [65, r], row 64 = softmax denominator
  A^T normalized via selector-matmul broadcast of 1/denom
  out = A Wo + bo          [512, 1024]    (A^T stationary, Wo moving)
"""

import sys
import os

if "/opt/trn_rl_repo" not in sys.path:
    sys.path.insert(0, "/opt/trn_rl_repo")

import numpy as np

HEADS = 16
DH = 64
B = 4
N = 1024
D = 1024
INNER = 1024
NCORES = 8
R = 512  # rows (query positions) per core
SCALE = DH ** -0.5  # 0.125

# matmul dtype: "bf16" (low power, FWL, 1 cyc/row), "f32r" (TF32-like,
# 1 cyc/row but power-throttles), or "f32" (exact, 4 cyc/row)
DTYPE_MODE = os.environ.get("BASS_ATTN_DTYPE", "bf16")

_CACHE = {}


def _build(dtype_mode: str, has_bias: bool = True):
    import concourse.bass as bass
    import concourse.mybir as mybir
    from concourse import bacc
    from concourse.tile import TileContext

    F32 = mybir.dt.float32
    # Tiles feeding the tensor engine are allocated in the matmul dtype so
    # producers (DMA/DVE/ACT) round on write, which the BIR verifier
    # requires for f32r and which enables FWL + low power for bf16.
    MDT = {"bf16": mybir.dt.bfloat16,
           "f32r": mybir.dt.float32r,
           "f32": mybir.dt.float32}[dtype_mode]

    def mm(ap):
        return ap

    nc = bacc.Bacc("TRN2", target_bir_lowering=False, debug=False,
                   num_devices=NCORES)

    xt = nc.dram_tensor("xt", [D, N], MDT, kind="ExternalInput")
    wq = nc.dram_tensor("wq", [D, INNER], MDT, kind="ExternalInput")
    wkv = nc.dram_tensor("wkv", [D, 2 * INNER], MDT, kind="ExternalInput")
    wo = nc.dram_tensor("wo", [INNER, D], MDT, kind="ExternalInput")
    bo = nc.dram_tensor("bo", [1, D], MDT, kind="ExternalInput")
    # cos/sin in the matmul dtype: 2-byte operands let the rotary
    # tensor_tensor ops hit the DVE 2x/4x perf modes
    cosk = nc.dram_tensor("cosk", [128, N], MDT, kind="ExternalInput")
    sink = nc.dram_tensor("sink", [128, N], MDT, kind="ExternalInput")
    sel = nc.dram_tensor("sel", [2, 128], MDT, kind="ExternalInput")
    ones1 = nc.dram_tensor("ones1", [1, 128], MDT, kind="ExternalInput")
    out = nc.dram_tensor("out", [R, D], F32, kind="ExternalOutput")

    KC = D // 128  # 8 contraction chunks of 128
    VW = HEADS * (DH + 1)  # 1040: v columns with a ones column per head

    with TileContext(nc) as tc:
        with tc.tile_pool(name="persist", bufs=1) as persist:
            # Persistent SBUF tensors (live across phases):
            # qt 16K + kt 32K + v 36K = 84KB/partition
            qt_sb = persist.tile([128, KC, R], MDT)          # qT (rotated)
            kt_sb = persist.tile([128, KC, N], MDT)          # kT (rotated)
            v_sb = persist.tile([128, KC, VW], MDT)          # v + ones cols

            # ones columns of v (column DH of each head's 65-wide group);
            # f32r memset fails the ISA check, so memset via an f32-typed
            # view there (1.0 is exact so no rounding is needed)
            vv = v_sb.rearrange("p c (h e) -> p c h e", e=DH + 1)
            ones_col = vv[:, :, :, DH:DH + 1]
            if MDT == mybir.dt.float32r:
                ones_col = ones_col.bitcast(F32)
            nc.vector.memset(ones_col, 1.0)

            # ---------------- Phase 1: QKV projections + rotary ----------
            # SBUF: xt 32K + cos 4K + sin 4K + weights 2x32K = 104KB
            with tc.tile_pool(name="ph1", bufs=1) as ph1, \
                 tc.tile_pool(name="wpool", bufs=2) as wpool, \
                 tc.tile_pool(name="rot_tmp", bufs=2) as rot_tmp, \
                 tc.tile_pool(name="ps_qkv", bufs=6, space="PSUM") as ps_qkv:

                xt_sb = ph1.tile([128, KC, N], MDT)
                cos_sb = ph1.tile([128, N], MDT)
                sin_sb = ph1.tile([128, N], MDT)
                # chunked loads, own-half columns first so the q matmuls
                # (which only read xt[:, 0:512]) start after ~2us; spread
                # across engine DMA queues so they run in parallel
                wq_sb = wpool.tile([128, KC, INNER], MDT, tag="w")
                for half in range(2):
                    lo, hi = half * 512, (half + 1) * 512
                    for k in range(KC):
                        nc.scalar.dma_start(
                            out=xt_sb[:, k, lo:hi],
                            in_=xt[k * 128:(k + 1) * 128, lo:hi])
                        if half == 0:
                            nc.sync.dma_start(
                                out=wq_sb[:, k, lo:hi],
                                in_=wq[k * 128:(k + 1) * 128, lo:hi])
                            nc.sync.dma_start(
                                out=wq_sb[:, k, hi:hi + 512],
                                in_=wq[k * 128:(k + 1) * 128, hi:hi + 512])
                    nc.scalar.dma_start(out=cos_sb[:, lo:hi],
                                        in_=cosk[:, lo:hi])
                    nc.scalar.dma_start(out=sin_sb[:, lo:hi],
                                        in_=sink[:, lo:hi])

                # swap adjacent partitions (2i <-> 2i+1) per 32-lane group;
                # the rotate-half sign lives in the host-signed sine input
                SWAP_MASK = [i ^ 1 for i in range(32)]
                rot_n = [0]

                def rotary(dst, cos_slc, sin_slc):
                    """dst (sbuf [128, 512]) holds fresh pre-rotary values;
                    apply dst = dst*cos + shuffle(dst)*sin_signed in place."""
                    rot_n[0] += 1
                    rt = rot_tmp.tile([128, 512], MDT,
                                      name=f"rt{rot_n[0]}", tag="rt")
                    nc.vector.stream_shuffle(rt[:], dst, mask=SWAP_MASK)
                    nc.vector.tensor_mul(rt[:], rt[:], sin_slc)
                    nc.vector.tensor_mul(dst, dst, cos_slc)
                    nc.vector.tensor_add(dst, dst, rt[:])

                # qT: out chunk c = [128 inner-cols, 512 own rows]
                for c in range(KC):
                    q_ps = ps_qkv.tile([128, 512], F32, tag="ps")
                    for k in range(KC):
                        nc.tensor.matmul(
                            q_ps[:],
                            mm(wq_sb[:, k, c * 128:(c + 1) * 128]),
                            mm(xt_sb[:, k, 0:R]),
                            start=(k == 0), stop=(k == KC - 1))
                    nc.scalar.copy(out=qt_sb[:, c, :], in_=q_ps[:])
                    rotary(qt_sb[:, c, :], cos_sb[:, 0:R], sin_sb[:, 0:R])

                # kT: out chunk c = [128 inner-cols, 1024 seq], 2 n-halves
                wk_sb = wpool.tile([128, KC, INNER], MDT, tag="w")
                for k in range(KC):
                    nc.sync.dma_start(
                        out=wk_sb[:, k, :],
                        in_=wkv[k * 128:(k + 1) * 128, 0:INNER])
                for c in range(KC):
                    for jh in range(2):
                        k_ps = ps_qkv.tile([128, 512], F32, tag="ps")
                        for k in range(KC):
                            nc.tensor.matmul(
                                k_ps[:],
                                mm(wk_sb[:, k, c * 128:(c + 1) * 128]),
                                mm(xt_sb[:, k, jh * 512:(jh + 1) * 512]),
                                start=(k == 0), stop=(k == KC - 1))
                        dst = kt_sb[:, c, jh * 512:(jh + 1) * 512]
                        nc.scalar.copy(out=dst, in_=k_ps[:])
                        rotary(dst, cos_sb[:, jh * 512:(jh + 1) * 512],
                               sin_sb[:, jh * 512:(jh + 1) * 512])

                # v: normal layout [seq, inner]; chunk m = seq rows,
                # n-half = 512 inner cols = 8 heads
                wv_sb = wpool.tile([128, KC, INNER], MDT, tag="w")
                for k in range(KC):
                    nc.sync.dma_start(
                        out=wv_sb[:, k, :],
                        in_=wkv[k * 128:(k + 1) * 128, INNER:2 * INNER])
                for m in range(KC):
                    for nh in range(2):
                        v_ps = ps_qkv.tile([128, 512], F32, tag="ps")
                        for k in range(KC):
                            nc.tensor.matmul(
                                v_ps[:],
                                mm(xt_sb[:, k, m * 128:(m + 1) * 128]),
                                mm(wv_sb[:, k, nh * 512:(nh + 1) * 512]),
                                start=(k == 0), stop=(k == KC - 1))
                        # scatter 8 heads of 64 cols into 65-strided groups
                        dst = vv[:, m, nh * 8:(nh + 1) * 8, 0:DH]
                        src = v_ps[:].rearrange("p (h e) -> p h e", e=DH)
                        nc.scalar.copy(out=dst, in_=src)

            # ------------- Phase 2: attention (2-deep pipelined) ----------
            # Heads are software-pipelined: scores for head h+2 are emitted
            # before PV of head h so the PE never waits on ACT's exp latency.
            # Softmax denominators are handled per head-pair: staged to
            # partition 0, bounced through DRAM into a [2, 512] tile, then a
            # K=2 selector matmul broadcasts 1/denom over the pair's 128
            # partitions and A^T is normalized in-loop.
            with tc.tile_pool(name="pt", bufs=3) as pt_pool, \
                 tc.tile_pool(name="ph2", bufs=1) as ph2, \
                 tc.tile_pool(name="stg", bufs=2) as stg_pool, \
                 tc.tile_pool(name="drp", bufs=2) as dr_pool, \
                 tc.tile_pool(name="wo_pool", bufs=2) as wo_pool, \
                 tc.tile_pool(name="out_st", bufs=4) as out_st:

                at_sb = ph2.tile([128, KC, R], MDT)  # A^T (attn out)
                # selector rows on partition 0 (same base partition as the
                # staged denominators, required by the matmul operand check)
                sel_sb = ph2.tile([1, 2, 128], MDT)
                bo_sb = ph2.tile([1, D], MDT)
                ones1_sb = ph2.tile([1, 128], MDT)
                nc.sync.dma_start(out=sel_sb[0:1, :, :],
                                  in_=sel[:, :].unsqueeze(0))
                nc.sync.dma_start(out=bo_sb[:], in_=bo[:, :])
                nc.sync.dma_start(out=ones1_sb[:], in_=ones1[:, :])

                # prefetch both Wo halves during the head loop
                wo_tiles = []
                for n in range(2):
                    wo_sb = wo_pool.tile([128, KC, 512], MDT,
                                         name=f"wo{n}", tag="woh")
                    nc.sync.dma_start(
                        out=wo_sb[:],
                        in_=wo[:, n * 512:(n + 1) * 512].rearrange(
                            "(c p) m -> p c m", p=128))
                    wo_tiles.append(wo_sb)

                with tc.tile_pool(name="ps_s", bufs=2, space="PSUM") as ps_s, \
                     tc.tile_pool(name="ps_pv", bufs=2,
                                  space="PSUM") as ps_pv:

                    pts = {}
                    stages = {}

                    def scores(h):
                        """scoresT + exp for head h -> pts[h]."""
                        c = h // 2
                        po = (h % 2) * 64
                        pt = pt_pool.tile([128, KC, R], MDT, name=f"pt{h}", tag="pt")
                        pts[h] = pt
                        for jp in range(KC // 2):  # pairs of j-chunks
                            s_ps = ps_s.tile([128, 2, 512], F32, tag="s")
                            for jj in range(2):
                                j = 2 * jp + jj
                                nc.tensor.matmul(
                                    s_ps[:, jj, :],
                                    mm(kt_sb[po:po + 64, c,
                                             j * 128:(j + 1) * 128]),
                                    mm(qt_sb[po:po + 64, c, :]),
                                    start=True, stop=True)
                            nc.scalar.activation(
                                out=pt[:, 2 * jp:2 * jp + 2, :], in_=s_ps[:],
                                func=mybir.ActivationFunctionType.Exp,
                                scale=SCALE)

                    def pv(h):
                        """PV for head h + denominator staging + per-pair
                        normalization of A^T."""
                        c = h // 2
                        po = (h % 2) * 64
                        pt = pts.pop(h)
                        pv_ps = ps_pv.tile([128, 512], F32, tag="pv")
                        for j in range(KC):
                            nc.tensor.matmul(
                                pv_ps[0:DH + 1, :],
                                mm(v_sb[:, j,
                                        h * (DH + 1):(h + 1) * (DH + 1)]),
                                mm(pt[:, j, :]),
                                start=(j == 0), stop=(j == KC - 1))
                        nc.vector.tensor_copy(at_sb[po:po + 64, c, :],
                                              pv_ps[0:DH, :])
                        if h % 2 == 0:
                            stages[c] = stg_pool.tile([1, 4, R], F32,
                                                      name=f"stg{c}", tag="stg")
                        # denom row: psum partition 64 -> partition 0 staging
                        # (reciprocal_approx_fast requires base partition 0)
                        nc.vector.tensor_copy(
                            stages[c][0:1, h % 2, :],
                            pv_ps[DH:DH + 1, :])
                        if h % 2 == 1:
                            # pair complete: one fast-approx reciprocal over
                            # both heads' denoms (flat 2D AP), cast to matmul
                            # dtype, broadcast each head's 1/denom over its
                            # 64 partitions via two accumulating K=1 selector
                            # matmuls, normalize chunk c
                            stage = stages.pop(c)
                            nc.vector.reciprocal_approx_fast(
                                stage[0:1, 2:4, :].rearrange(
                                    "p a b -> p (a b)"),
                                stage[0:1, 0:2, :].rearrange(
                                    "p a b -> p (a b)"))
                            drb = dr_pool.tile([1, 2, R], MDT,
                                               name=f"dr{c}", tag="dr")
                            nc.vector.tensor_copy(drb[0:1, :, :],
                                                  stage[0:1, 2:4, :])
                            b_ps = ps_pv.tile([128, 512], F32, tag="b")
                            nc.tensor.matmul(
                                b_ps[:],
                                mm(sel_sb[0:1, 0, :]),
                                mm(drb[0:1, 0, :]),
                                start=True, stop=False)
                            nc.tensor.matmul(
                                b_ps[:],
                                mm(sel_sb[0:1, 1, :]),
                                mm(drb[0:1, 1, :]),
                                start=False, stop=True)
                            nc.vector.tensor_mul(at_sb[:, c, :],
                                                 at_sb[:, c, :], b_ps[:])

                    scores(0)
                    scores(1)
                    for h in range(HEADS):
                        if h + 2 < HEADS:
                            scores(h + 2)
                        pv(h)

                # ------------- Phase 3: output projection ----------------
                with tc.tile_pool(name="ps_f", bufs=4, space="PSUM") as ps_f:
                    for n in range(2):
                        wo_sb = wo_tiles[n]
                        for m in range(4):
                            f_ps = ps_f.tile([128, 512], F32)
                            for k in range(KC):
                                nc.tensor.matmul(
                                    f_ps[:],
                                    mm(at_sb[:, k, m * 128:(m + 1) * 128]),
                                    mm(wo_sb[:, k, :]),
                                    start=(k == 0),
                                    stop=(not has_bias and k == KC - 1))
                            if has_bias:
                                nc.tensor.matmul(
                                    f_ps[:], mm(ones1_sb[:]),
                                    mm(bo_sb[0:1, n * 512:(n + 1) * 512]),
                                    start=False, stop=True)
                            o_sb = out_st.tile([128, 512], F32)
                            nc.scalar.copy(out=o_sb[:], in_=f_ps[:])
                            nc.sync.dma_start(
                                out=out[m * 128:(m + 1) * 128,
                                        n * 512:(n + 1) * 512],
                                in_=o_sb[:])

    nc.compile()
    return nc


def _host_prep(x, rotary_emb, Wq, Wkv, Wo, bo, dtype_mode="f32"):
    """Build the per-core input maps."""
    if dtype_mode == "bf16":
        import ml_dtypes
        mnp = ml_dtypes.bfloat16
    else:
        mnp = np.float32
    x = np.asarray(x, dtype=np.float32)
    rotary_emb = np.asarray(rotary_emb, dtype=np.float32)
    Wq = np.ascontiguousarray(np.asarray(Wq, dtype=np.float32))
    Wkv = np.ascontiguousarray(np.asarray(Wkv, dtype=np.float32))
    Wo = np.ascontiguousarray(np.asarray(Wo, dtype=np.float32))
    bo_row = np.ascontiguousarray(np.asarray(bo, dtype=np.float32)[None, :])

    cosT = np.cos(rotary_emb).T.astype(np.float32)  # [64, 1024]
    sinT = np.sin(rotary_emb).T.astype(np.float32)
    cos2 = np.concatenate([cosT, cosT], axis=0)  # [128, n]
    sin2 = np.concatenate([sinT, sinT], axis=0)
    # rotate-half sign: rot[2i] = -x[2i+1], rot[2i+1] = +x[2i]; the device
    # only swaps lanes, so bake the sign into the sine rows
    sign = np.where(np.arange(128) % 2 == 0, -1.0, 1.0).astype(np.float32)
    sin2 = sin2 * sign[:, None]

    # selector rows: head-even -> partitions 0..63, head-odd -> 64..127
    sel = np.zeros((2, 128), dtype=np.float32)
    sel[0, 0:64] = 1.0
    sel[1, 64:128] = 1.0

    ones1 = np.ones((1, 128), dtype=np.float32)

    in_maps = []
    for core in range(NCORES):
        b, half = divmod(core, 2)
        perm = np.concatenate([
            np.arange(half * R, (half + 1) * R),
            np.arange((1 - half) * R, (2 - half) * R)])
        xt = np.ascontiguousarray(x[b].T[:, perm])  # [D, N] own half first
        in_maps.append({
            "xt": xt.astype(mnp),
            "wq": Wq.astype(mnp),
            "wkv": Wkv.astype(mnp),
            "wo": Wo.astype(mnp),
            "bo": bo_row.astype(mnp),
            "cosk": np.ascontiguousarray(cos2[:, perm]).astype(mnp),
            "sink": np.ascontiguousarray(sin2[:, perm]).astype(mnp),
            "sel": sel.astype(mnp),
            "ones1": ones1.astype(mnp),
        })
    return in_maps


def _run(inputs, trace=False, trace_cores=None):
    from concourse.bass_utils import run_bass_kernel_spmd

    has_bias = bool(np.any(np.asarray(inputs["bo"])))
    key = ("nc", DTYPE_MODE, has_bias)
    if key not in _CACHE:
        _CACHE[key] = _build(DTYPE_MODE, has_bias=has_bias)
    nc = _CACHE[key]

    in_maps = _host_prep(dtype_mode=DTYPE_MODE, **inputs)
    res = run_bass_kernel_spmd(nc, in_maps, list(range(NCORES)),
                               trace=trace, trace_cores=trace_cores)
    out = np.empty((B, N, D), dtype=np.float32)
    for core in range(NCORES):
        b, half = divmod(core, 2)
        out[b, half * R:(half + 1) * R, :] = res.results[core]["out"]
    return out, res


def kernel(**inputs):
    out, _ = _run(inputs, trace=False)
    return out



# revision 26
# speedup vs baseline: 1.5301x; 1.1134x over previous
"""Self-contained Trainium2 Bass kernel for nn_Attention_26740466385723.

Full-input contract: kernel(**inputs) takes the unsharded numpy inputs and
returns the full [4, 1024, 1024] output.

Sharding (zero-collective): 8 cores = 4 batch x 2 sequence-halves. Each core
computes the attention output rows for half the sequence of one batch element.
The KV projection is duplicated within each batch pair (33% extra flops) which
avoids any inter-core collective.

v2: single fully-interleaved schedule. Engine queues execute in emission
order; the emission order software-pipelines all phases so the PE stream is
dense and the ACT exp stream starts ~25us into the kernel:

  PE:   q (k-major over 8 psum banks) | per c: kT(c) -> scores(pair c-1) ->
        v-block -> PV(pair c-2) -> selector+norm(pair c-3) | tail | out-proj
  ACT:  q/k psum->sbuf copies + the exp stream (the attention floor)
  DVE:  rotary (bf16 fast mode), v scatter copies, A^T copies, denom
        staging, approx-reciprocal, casts, normalize muls
  GPSIMD queue: xt own-half + cos/sin direct DMAs
  Scalar queue: xt other-half direct DMAs (done before the first ACT copy)
  Sync ring: sel/bo/ones + weights wq -> wk -> wv -> wo (async HWDGE)

Per-core device algorithm (layouts chosen so no on-device transpose is ever
needed):
  qT  = Wq^T x_own^T        [inner, 512]   (Wq stationary, xT moving)
  kT  = Wk^T x_all^T        [inner, 1024]
  v   = x_all Wv            [1024, inner]  (xT stationary, Wv moving)
  rotary on qT/kT: qrotT = qT*cos + shuffle(qT)*sin_signed (DVE lane swap)
  per head h: scoresT[j, r] = sum_d kT[d,j] qT[d,r]
              P^T = exp(scoresT * 0.125)   (no max subtraction; scores~N(0,1))
              pv  = [v_h | 1]^T @ P^T -> [65, r], row 64 = softmax denominator
  A^T normalized via two K=1 selector matmuls broadcasting 1/denom
  out = A Wo + bo          [512, 1024]    (A^T stationary, Wo moving)
"""

import sys
import os

if "/opt/trn_rl_repo" not in sys.path:
    sys.path.insert(0, "/opt/trn_rl_repo")

import numpy as np

HEADS = 16
DH = 64
B = 4
N = 1024
D = 1024
INNER = 1024
NCORES = 8
R = 512  # rows (query positions) per core
SCALE = DH ** -0.5  # 0.125

# matmul dtype: "bf16" (low power, FWL, 1 cyc/row), "f32r" (TF32-like,
# 1 cyc/row but power-throttles), or "f32" (exact, 4 cyc/row)
DTYPE_MODE = os.environ.get("BASS_ATTN_DTYPE", "bf16")

_CACHE = {}


def _build(dtype_mode: str, has_bias: bool = True):
    import concourse.bass as bass
    import concourse.mybir as mybir
    from concourse import bacc
    from concourse.tile import TileContext

    F32 = mybir.dt.float32
    MDT = {"bf16": mybir.dt.bfloat16,
           "f32r": mybir.dt.float32r,
           "f32": mybir.dt.float32}[dtype_mode]

    def mm(ap):
        return ap

    nc = bacc.Bacc("TRN2", target_bir_lowering=False, debug=False,
                   num_devices=NCORES)

    xt = nc.dram_tensor("xt", [D, N], MDT, kind="ExternalInput")
    wq = nc.dram_tensor("wq", [D, INNER], MDT, kind="ExternalInput")
    wkv = nc.dram_tensor("wkv", [D, 2 * INNER], MDT, kind="ExternalInput")
    wo = nc.dram_tensor("wo", [INNER, D], MDT, kind="ExternalInput")
    bo = nc.dram_tensor("bo", [1, D], MDT, kind="ExternalInput")
    cosk = nc.dram_tensor("cosk", [128, N], MDT, kind="ExternalInput")
    sink = nc.dram_tensor("sink", [128, N], MDT, kind="ExternalInput")
    sel = nc.dram_tensor("sel", [2, 128], MDT, kind="ExternalInput")
    ones1 = nc.dram_tensor("ones1", [1, 128], MDT, kind="ExternalInput")
    out = nc.dram_tensor("out", [R, D], F32, kind="ExternalOutput")

    KC = D // 128  # 8 contraction chunks of 128
    VW = HEADS * (DH + 1)  # 1040: v columns with a ones column per head

    with TileContext(nc) as tc:
        with tc.tile_pool(name="persist", bufs=1) as persist, \
             tc.tile_pool(name="wpool", bufs=3) as wpool, \
             tc.tile_pool(name="wo_pool", bufs=2) as wo_pool, \
             tc.tile_pool(name="pt", bufs=4) as pt_pool, \
             tc.tile_pool(name="rot_tmp", bufs=2) as rot_tmp, \
             tc.tile_pool(name="stg", bufs=2) as stg_pool, \
             tc.tile_pool(name="drp", bufs=2) as dr_pool, \
             tc.tile_pool(name="out_st", bufs=4) as out_st:

            qt_sb = persist.tile([128, KC, R], MDT)          # qT (rotated)
            kt_sb = persist.tile([128, KC, N], MDT)          # kT (rotated)
            v_sb = persist.tile([128, KC, VW], MDT)          # v + ones cols
            at_sb = persist.tile([128, KC, R], MDT)          # A^T (attn out)
            xt_sb = persist.tile([128, KC, N], MDT)
            cos_sb = persist.tile([128, N], MDT)
            sin_sb = persist.tile([128, N], MDT)
            sel_sb = persist.tile([1, 2, 128], MDT)
            bo_sb = persist.tile([1, D], MDT)
            ones1_sb = persist.tile([1, 128], MDT)

            # ones columns of v (column DH of each head's 65-wide group)
            vv = v_sb.rearrange("p c (h e) -> p c h e", e=DH + 1)
            ones_col = vv[:, :, :, DH:DH + 1]
            if MDT == mybir.dt.float32r:
                ones_col = ones_col.bitcast(F32)
            nc.vector.memset(ones_col, 1.0)

            # ---------------- DMA emission ------------------------------
            # gpsimd queue (direct): xt own half in k order, then trig
            for k in range(KC):
                nc.gpsimd.dma_start(out=xt_sb[:, k, 0:R],
                                    in_=xt[k * 128:(k + 1) * 128, 0:R])
            nc.gpsimd.dma_start(out=cos_sb[:, 0:R], in_=cosk[:, 0:R])
            nc.gpsimd.dma_start(out=sin_sb[:, 0:R], in_=sink[:, 0:R])
            nc.gpsimd.dma_start(out=cos_sb[:, R:N], in_=cosk[:, R:N])
            nc.gpsimd.dma_start(out=sin_sb[:, R:N], in_=sink[:, R:N])
            # scalar queue (direct): xt other half; finishes before the
            # first ACT copy is due
            for k in range(KC):
                nc.scalar.dma_start(out=xt_sb[:, k, R:N],
                                    in_=xt[k * 128:(k + 1) * 128, R:N])
            # sync ring (async): small tensors then weights in use order
            nc.sync.dma_start(out=sel_sb[0:1, :, :],
                              in_=sel[:, :].unsqueeze(0))
            nc.sync.dma_start(out=bo_sb[:], in_=bo[:, :])
            nc.sync.dma_start(out=ones1_sb[:], in_=ones1[:, :])
            wq_sb = wpool.tile([128, KC, INNER], MDT, tag="w", name="wq")
            wk_sb = wpool.tile([128, KC, INNER], MDT, tag="w", name="wk")
            wv_sb = wpool.tile([128, KC, INNER], MDT, tag="w", name="wv")
            for k in range(KC):
                nc.sync.dma_start(out=wq_sb[:, k, :],
                                  in_=wq[k * 128:(k + 1) * 128, :])
            for k in range(KC):
                nc.sync.dma_start(out=wk_sb[:, k, :],
                                  in_=wkv[k * 128:(k + 1) * 128, 0:INNER])
            for k in range(KC):
                nc.sync.dma_start(out=wv_sb[:, k, :],
                                  in_=wkv[k * 128:(k + 1) * 128,
                                          INNER:2 * INNER])
            wo_tiles = []
            for n in range(2):
                wo_sb = wo_pool.tile([128, KC, 512], MDT,
                                     name=f"wo{n}", tag="woh")
                nc.sync.dma_start(
                    out=wo_sb[:],
                    in_=wo[:, n * 512:(n + 1) * 512].rearrange(
                        "(c p) m -> p c m", p=128))
                wo_tiles.append(wo_sb)

            # ---------------- rotary helper (DVE) -----------------------
            # swap adjacent partitions (2i <-> 2i+1) per 32-lane group;
            # the rotate-half sign lives in the host-signed sine input
            SWAP_MASK = [i ^ 1 for i in range(32)]
            rot_n = [0]

            def rotary(dst, cos_slc, sin_slc):
                rot_n[0] += 1
                rt = rot_tmp.tile([128, 512], MDT,
                                  name=f"rt{rot_n[0]}", tag="rt")
                nc.vector.stream_shuffle(rt[:], dst, mask=SWAP_MASK)
                nc.vector.tensor_mul(rt[:], rt[:], sin_slc)
                nc.vector.tensor_mul(dst, dst, cos_slc)
                nc.vector.tensor_add(dst, dst, rt[:])

            # ---------------- q projection (k-major, 8 banks) -----------
            with tc.tile_pool(name="ps_q", bufs=8, space="PSUM") as ps_q:
                qps = [ps_q.tile([128, 512], F32, name=f"q{c}", tag="q")
                       for c in range(KC)]
                for k in range(KC):
                    for c in range(KC):
                        nc.tensor.matmul(
                            qps[c][:],
                            mm(wq_sb[:, k, c * 128:(c + 1) * 128]),
                            mm(xt_sb[:, k, 0:R]),
                            start=(k == 0), stop=(k == KC - 1))
                for c in range(KC):
                    nc.scalar.copy(out=qt_sb[:, c, :], in_=qps[c][:])
                    rotary(qt_sb[:, c, :], cos_sb[:, 0:R], sin_sb[:, 0:R])

            # ---------------- main interleaved loop ---------------------
            with tc.tile_pool(name="ps_kv", bufs=4, space="PSUM") as ps_kv, \
                 tc.tile_pool(name="ps_s", bufs=2, space="PSUM") as ps_s:

                pts = {}
                stages = {}

                def emit_k(c):
                    for jh in range(2):
                        kp = ps_kv.tile([128, 512], F32, tag="kv",
                                        name=f"k{c}_{jh}")
                        for k in range(KC):
                            nc.tensor.matmul(
                                kp[:],
                                mm(wk_sb[:, k, c * 128:(c + 1) * 128]),
                                mm(xt_sb[:, k, jh * 512:(jh + 1) * 512]),
                                start=(k == 0), stop=(k == KC - 1))
                        dst = kt_sb[:, c, jh * 512:(jh + 1) * 512]
                        nc.scalar.copy(out=dst, in_=kp[:])
                        rotary(dst, cos_sb[:, jh * 512:(jh + 1) * 512],
                               sin_sb[:, jh * 512:(jh + 1) * 512])

                def emit_v(m, nh):
                    vp = ps_kv.tile([128, 512], F32, tag="kv",
                                    name=f"v{m}_{nh}")
                    for k in range(KC):
                        nc.tensor.matmul(
                            vp[:],
                            mm(xt_sb[:, k, m * 128:(m + 1) * 128]),
                            mm(wv_sb[:, k, nh * 512:(nh + 1) * 512]),
                            start=(k == 0), stop=(k == KC - 1))
                    dst = vv[:, m, nh * 8:(nh + 1) * 8, 0:DH]
                    nc.vector.tensor_copy(
                        dst, vp[:].rearrange("p (h e) -> p h e", e=DH))

                def emit_scores(h):
                    c = h // 2
                    po = (h % 2) * 64
                    pt = pt_pool.tile([128, KC, R], MDT,
                                      name=f"pt{h}", tag="pt")
                    pts[h] = pt
                    for jp in range(KC // 2):
                        s_ps = ps_s.tile([128, 2, 512], F32, tag="s")
                        for jj in range(2):
                            j = 2 * jp + jj
                            nc.tensor.matmul(
                                s_ps[:, jj, :],
                                mm(kt_sb[po:po + 64, c,
                                         j * 128:(j + 1) * 128]),
                                mm(qt_sb[po:po + 64, c, :]),
                                start=True, stop=True)
                        nc.scalar.activation(
                            out=pt[:, 2 * jp:2 * jp + 2, :], in_=s_ps[:],
                            func=mybir.ActivationFunctionType.Exp,
                            scale=SCALE)

                def emit_pv(h):
                    c = h // 2
                    po = (h % 2) * 64
                    pt = pts.pop(h)
                    pv_ps = ps_kv.tile([128, 512], F32, tag="kv",
                                       name=f"pv{h}")
                    for j in range(KC):
                        nc.tensor.matmul(
                            pv_ps[0:DH + 1, :],
                            mm(v_sb[:, j, h * (DH + 1):(h + 1) * (DH + 1)]),
                            mm(pt[:, j, :]),
                            start=(j == 0), stop=(j == KC - 1))
                    nc.vector.tensor_copy(at_sb[po:po + 64, c, :],
                                          pv_ps[0:DH, :])
                    if h % 2 == 0:
                        stages[c] = stg_pool.tile([1, 4, R], F32,
                                                  name=f"stg{c}", tag="stg")
                    # denom row: psum partition 64 -> partition 0 staging
                    # (reciprocal_approx_fast requires base partition 0)
                    nc.vector.tensor_copy(stages[c][0:1, h % 2, :],
                                          pv_ps[DH:DH + 1, :])

                def emit_norm(c):
                    # one fast-approx reciprocal over the pair's denoms,
                    # cast to matmul dtype, broadcast each head's 1/denom
                    # over its 64 partitions via two accumulating K=1
                    # selector matmuls, normalize chunk c of A^T
                    stage = stages.pop(c)
                    nc.vector.reciprocal_approx_fast(
                        stage[0:1, 2:4, :].rearrange("p a b -> p (a b)"),
                        stage[0:1, 0:2, :].rearrange("p a b -> p (a b)"))
                    drb = dr_pool.tile([1, 2, R], MDT,
                                       name=f"dr{c}", tag="dr")
                    nc.vector.tensor_copy(drb[0:1, :, :],
                                          stage[0:1, 2:4, :])
                    b_ps = ps_kv.tile([128, 512], F32, tag="kv",
                                      name=f"b{c}")
                    nc.tensor.matmul(
                        b_ps[:], mm(sel_sb[0:1, 0, :]), mm(drb[0:1, 0, :]),
                        start=True, stop=False)
                    nc.tensor.matmul(
                        b_ps[:], mm(sel_sb[0:1, 1, :]), mm(drb[0:1, 1, :]),
                        start=False, stop=True)
                    nc.vector.tensor_mul(at_sb[:, c, :],
                                         at_sb[:, c, :], b_ps[:])

                # schedule: scores lag k by 1 chunk, pv lags scores by 1,
                # norm lags pv by 1; v-blocks fill the early iterations
                for c in range(KC):
                    emit_k(c)
                    if c >= 1:
                        emit_scores(2 * (c - 1))
                        emit_scores(2 * (c - 1) + 1)
                    if c < 4:
                        nh, mg = divmod(c, 2)
                        for m in range(4 * mg, 4 * mg + 4):
                            emit_v(m, nh)
                    if c >= 2:
                        emit_pv(2 * (c - 2))
                        emit_pv(2 * (c - 2) + 1)
                    if c >= 3:
                        emit_norm(c - 3)
                # tail
                emit_scores(14)
                emit_scores(15)
                emit_pv(12)
                emit_pv(13)
                emit_norm(5)
                emit_pv(14)
                emit_pv(15)
                emit_norm(6)
                emit_norm(7)

            # ---------------- output projection -------------------------
            with tc.tile_pool(name="ps_f", bufs=4, space="PSUM") as ps_f:
                for n in range(2):
                    wo_sb = wo_tiles[n]
                    for m in range(4):
                        f_ps = ps_f.tile([128, 512], F32)
                        for k in range(KC):
                            nc.tensor.matmul(
                                f_ps[:],
                                mm(at_sb[:, k, m * 128:(m + 1) * 128]),
                                mm(wo_sb[:, k, :]),
                                start=(k == 0),
                                stop=(not has_bias and k == KC - 1))
                        if has_bias:
                            nc.tensor.matmul(
                                f_ps[:], mm(ones1_sb[:]),
                                mm(bo_sb[0:1, n * 512:(n + 1) * 512]),
                                start=False, stop=True)
                        o_sb = out_st.tile([128, 512], F32)
                        nc.scalar.copy(out=o_sb[:], in_=f_ps[:])
                        nc.sync.dma_start(
                            out=out[m * 128:(m + 1) * 128,
                                    n * 512:(n + 1) * 512],
                            in_=o_sb[:])

    nc.compile()
    return nc


def _host_prep(x, rotary_emb, Wq, Wkv, Wo, bo, dtype_mode="f32"):
    """Build the per-core input maps."""
    if dtype_mode == "bf16":
        import ml_dtypes
        mnp = ml_dtypes.bfloat16
    else:
        mnp = np.float32
    x = np.asarray(x, dtype=np.float32)
    rotary_emb = np.asarray(rotary_emb, dtype=np.float32)
    Wq = np.ascontiguousarray(np.asarray(Wq, dtype=np.float32))
    Wkv = np.ascontiguousarray(np.asarray(Wkv, dtype=np.float32))
    Wo = np.ascontiguousarray(np.asarray(Wo, dtype=np.float32))
    bo_row = np.ascontiguousarray(np.asarray(bo, dtype=np.float32)[None, :])

    cosT = np.cos(rotary_emb).T.astype(np.float32)  # [64, 1024]
    sinT = np.sin(rotary_emb).T.astype(np.float32)
    cos2 = np.concatenate([cosT, cosT], axis=0)  # [128, n]
    sin2 = np.concatenate([sinT, sinT], axis=0)
    # rotate-half sign: rot[2i] = -x[2i+1], rot[2i+1] = +x[2i]; the device
    # only swaps lanes, so bake the sign into the sine rows
    sign = np.where(np.arange(128) % 2 == 0, -1.0, 1.0).astype(np.float32)
    sin2 = sin2 * sign[:, None]

    # selector rows: head-even -> partitions 0..63, head-odd -> 64..127
    sel = np.zeros((2, 128), dtype=np.float32)
    sel[0, 0:64] = 1.0
    sel[1, 64:128] = 1.0

    ones1 = np.ones((1, 128), dtype=np.float32)

    in_maps = []
    for core in range(NCORES):
        b, half = divmod(core, 2)
        perm = np.concatenate([
            np.arange(half * R, (half + 1) * R),
            np.arange((1 - half) * R, (2 - half) * R)])
        xt = np.ascontiguousarray(x[b].T[:, perm])  # [D, N] own half first
        in_maps.append({
            "xt": xt.astype(mnp),
            "wq": Wq.astype(mnp),
            "wkv": Wkv.astype(mnp),
            "wo": Wo.astype(mnp),
            "bo": bo_row.astype(mnp),
            "cosk": np.ascontiguousarray(cos2[:, perm]).astype(mnp),
            "sink": np.ascontiguousarray(sin2[:, perm]).astype(mnp),
            "sel": sel.astype(mnp),
            "ones1": ones1.astype(mnp),
        })
    return in_maps


def _run(inputs, trace=False, trace_cores=None):
    from concourse.bass_utils import run_bass_kernel_spmd

    has_bias = bool(np.any(np.asarray(inputs["bo"])))
    key = ("nc", DTYPE_MODE, has_bias)
    if key not in _CACHE:
        _CACHE[key] = _build(DTYPE_MODE, has_bias=has_bias)
    nc = _CACHE[key]

    in_maps = _host_prep(dtype_mode=DTYPE_MODE, **inputs)
    res = run_bass_kernel_spmd(nc, in_maps, list(range(NCORES)),
                               trace=trace, trace_cores=trace_cores)
    out = np.empty((B, N, D), dtype=np.float32)
    for core in range(NCORES):
        b, half = divmod(core, 2)
        out[b, half * R:(half + 1) * R, :] = res.results[core]["out"]
    return out, res


def kernel(**inputs):
    out, _ = _run(inputs, trace=False)
    return out
